# revision 4
# baseline (speedup 1.0000x reference)
"""Trainium2 Bass kernel for nn_BuildLstmUnrollNet (2-layer LSTM, 11-step unroll,
per-step weights), distributed over 8 NeuronCores.

Strategy: 8-way tensor parallelism over the 4*R gate dimension. Each core owns a
128-row slice of each of the 4 gates (512 of 4096 pre-activation columns), so
per-step weight reads are fully disjoint across cores (HBM traffic /8). The
full batch (256) is kept on every core.

Matmul dataflow is h-STATIONARY: the stationary operand is an h^T k-chunk
[K=128, 128 batch-half] and the moving operand is the weight k-tile slice
[K=128, 512 gate-dims] (fp8, scaled x64; compensated by ACT scale=1/64).
Each 128x128 stationary load streams 512 moving columns (vs 256 in the
out^T-stationary layout), halving LDWEIGHTS pressure and instruction count:
56 matmuls/step instead of 120. PSUM holds pre = [batch-half, 512 gate-dims]
(one full bank), gating runs in [batch, gate-dim] layout, and the updated h
slice is PE-transposed back to [dim, batch] for the AllGather.

Each step needs two 8-way AllGathers of the updated h slices (h0 for the
layer-1 input + next step's Wh0, h1 for the next step's Wh1); they serialize
on the single per-core CC stream, so the schedule launches AG0(t) as soon as
gate0(t) + its transpose are done, and AG1(t-1) late-body. A tiny dummy
AllGather is issued first-thing so the runtime's one-time CC rendezvous
barrier (~40us) overlaps the prologue DMAs/matmuls instead of the first real
gather. h is bf16 end to end; y lands bf16 (upcast on the host); c stays fp32.
"""
import numpy as np

B, I, R, L, U = 256, 512, 1024, 2, 11
U_RUN = U - 1          # the 11th step never reaches the output
N_CORES = 8
RC = 128               # per-core rows per gate
W = 4 * RC             # per-core pre width (512)
NKX = I // 128         # 4 x k-tiles (layer0 input part)
NKH = R // 128         # 8 h k-tiles
HB = 128               # batch half
# per-step weight k-tile groups, in use order
OFF_WH0 = 0
OFF_WI1 = 8
OFF_WH1 = 16
OFF_WI0 = 24
NKALL = 28
# gate column offsets inside a pre tile [batch, W]: [i|f|o|g]
G_I, G_F, G_O, G_G = 0, 128, 256, 384


def build_program(reps: int = 1):
    import concourse.bacc as bacc
    import concourse.mybir as mybir
    import concourse.tile as tile
    from concourse import masks

    F32 = mybir.dt.float32
    BF16 = mybir.dt.bfloat16
    FP8 = mybir.dt.float8e4
    Sig = mybir.ActivationFunctionType.Sigmoid
    Tanh = mybir.ActivationFunctionType.Tanh

    PHASE_MARKS.clear()
    nc = bacc.Bacc("TRN2", target_bir_lowering=False, debug=False,
                   num_devices=N_CORES)

    xT_d = nc.dram_tensor("xT", [I, B], BF16, kind="ExternalInput")
    h0T_d = nc.dram_tensor("h0T0", [R, B], BF16, kind="ExternalInput")
    h1T_d = nc.dram_tensor("h1T0", [R, B], BF16, kind="ExternalInput")
    c0_d = nc.dram_tensor("c0i", [B, RC], F32, kind="ExternalInput")
    c1_d = nc.dram_tensor("c1i", [B, RC], F32, kind="ExternalInput")
    # weights in scaled fp8-e4m3 (x64; compensated by ACT scale=1/64):
    # halves the dominant HBM/DMA-engine traffic
    wall_d = nc.dram_tensor("wall", [U_RUN, 128, NKALL * W], FP8,
                            kind="ExternalInput")
    b0_d = nc.dram_tensor("b0", [1, U_RUN * W], BF16, kind="ExternalInput")
    b1_d = nc.dram_tensor("b1", [1, U_RUN * W], BF16, kind="ExternalInput")
    # y[s] = h1 slice in [batch, dim] orientation
    y_d = nc.dram_tensor("y", [U_RUN, B, RC], BF16, kind="ExternalOutput")

    T = reps * U_RUN

    with tile.TileContext(nc) as tc:
        with tc.tile_pool(name="const", bufs=1) as constp, \
             tc.tile_pool(name="h0pool", bufs=2) as h0pool, \
             tc.tile_pool(name="h1pool", bufs=2) as h1pool, \
             tc.tile_pool(name="cpool", bufs=2) as cpool, \
             tc.tile_pool(name="wpool", bufs=4) as wpool, \
             tc.tile_pool(name="gpool", bufs=2) as gpool, \
             tc.tile_pool(name="ps0", bufs=2, space="PSUM") as ps0, \
             tc.tile_pool(name="ps1", bufs=1, space="PSUM") as ps1, \
             tc.tile_pool(name="tppool", bufs=1, space="PSUM") as tppool, \
             tc.tile_pool(name="dramp", bufs=2, space="DRAM") as dramp:

            # ---- CC warm-up: tiny AllGather so the one-time rendezvous
            # barrier runs during the prologue, not before the first real AG.
            ones = constp.tile([1, 128], BF16)
            nc.gpsimd.memset(ones[:], 1.0)
            dum_in = dramp.tile([1, 128], BF16, name="dumin")
            nc.sync.dma_start(dum_in[:], ones[:])
            dum_out = dramp.tile([N_CORES, 128], BF16, name="dumout",
                                 addr_space="Shared")
            nc.gpsimd.collective_compute(
                "AllGather", mybir.AluOpType.bypass,
                replica_groups=[list(range(N_CORES))],
                ins=[dum_in[:]], outs=[dum_out[:]],
            )

            # ---- constants ----
            ident = constp.tile([128, 128], BF16)
            masks.make_identity(nc, ident[:])
            xT_tiles = []
            for kk in range(NKX):
                xt = constp.tile([128, B], BF16, name=f"xT_{kk}")
                nc.sync.dma_start(xt[:], xT_d.ap()[kk * 128 : (kk + 1) * 128, :])
                xT_tiles.append(xt)
            bias0 = constp.tile([1, U_RUN * W], BF16)
            nc.sync.dma_start(bias0[:], b0_d.ap())
            bias1 = constp.tile([1, U_RUN * W], BF16)
            nc.sync.dma_start(bias1[:], b1_d.ap())

            # ---- helpers ----
            def load_h(layer, src_ap, nchunk=4):
                """[R, B] DRAM h^T -> one [128, NKH*B] tile (k-chunk kk at
                columns [kk*B, (kk+1)*B)), chunked so early-k consumers start
                sooner."""
                pool = h0pool if layer == 0 else h1pool
                t = pool.tile([128, NKH * B], BF16, name=f"h{layer}T",
                              tag=f"h{layer}T")
                step = NKH // nchunk
                for lo in range(0, NKH, step):
                    nc.sync.dma_start(
                        t[:, lo * B : (lo + step) * B]
                        .rearrange("p (k b) -> p k b", k=step),
                        src_ap[lo * 128 : (lo + step) * 128, :]
                        .rearrange("(k p) b -> p k b", p=128),
                    )
                return t

            def hstat(h_s, kh, half):
                """stationary h^T chunk [128k, 128 batch-half]"""
                return h_s[:, kh * B + half * HB : kh * B + half * HB + HB]

            # 4-k-tile chunks -> 4KB per-partition runs (full DMA packets)
            W_CHUNKS_A = [("w", 0, 4), ("w", 4, 8), ("w", 8, 12)]
            W_CHUNKS_B = [("w", 12, 16), ("w", 16, 20), ("w", 20, 24),
                          ("w", 24, 28)]

            def alloc_wt():
                return wpool.tile([128, NKALL * W], FP8, name="w_s", tag="w_s")

            def load_wt_dmas(t, widx, chunks):
                src = wall_d.ap()[widx]
                for _, lo, hi in chunks:
                    nc.sync.dma_start(t[:, lo * W : hi * W],
                                      src[:, lo * W : hi * W])

            def load_wt(widx):
                t = alloc_wt()
                load_wt_dmas(t, widx, W_CHUNKS_A)
                load_wt_dmas(t, widx, W_CHUNKS_B)
                return t

            def wmov(wt, blk):
                """moving weight block [128k, W outdims]"""
                return wt[:, blk * W : (blk + 1) * W]

            def mm(out_ap, lhsT, rhs, start, stop):
                nc.tensor.matmul(out_ap, lhsT, rhs, start=start, stop=stop)

            # pre tiles: one [128 batch-half, W] PSUM tile (full bank) per half
            def alloc_pre(pool, tag):
                return [pool.tile([HB, W], F32, name=f"{tag}{h}",
                                  tag=f"{tag}{h}") for h in range(2)]

            def bias_mm(pre, bias_t, widx):
                """pre[half] = ones^T @ bias slice (start of accum group)"""
                for h in range(2):
                    mm(pre[h][:], ones[:], bias_t[:, widx * W : (widx + 1) * W],
                       True, False)

            def part_mms(pre, wt, off, h_s, half, stop):
                """one weight part (8 k-tiles) for one batch half."""
                for kh in range(NKH):
                    mm(pre[half][:], hstat(h_s, kh, half),
                       wmov(wt, off + kh), False, stop and kh == NKH - 1)

            def xpart_mms(pre, wt):
                for h in range(2):
                    for kx in range(NKX):
                        mm(pre[h][:],
                           xT_tiles[kx][:, h * HB : h * HB + HB],
                           wmov(wt, OFF_WI0 + kx), False, False)

            def gate_cells(layer, pre, half, c_old):
                """LSTM cell elementwise in [batch-half, dim] layout; returns
                (h bf16 [128, 128], c_new f32 [128, 128])."""
                p = pre[half][:]
                si = gpool.tile([HB, RC], F32, name=f"si{layer}{half}",
                                tag=f"si{layer}{half}")
                nc.scalar.activation(si[:], p[:, G_I:G_I + RC], Sig,
                                     scale=1.0 / 64.0)
                sf = gpool.tile([HB, RC], F32, name=f"sf{layer}{half}",
                                tag=f"sf{layer}{half}")
                nc.scalar.activation(sf[:], p[:, G_F:G_F + RC], Sig,
                                     scale=1.0 / 64.0)
                tg = gpool.tile([HB, RC], F32, name=f"tg{layer}{half}",
                                tag=f"tg{layer}{half}")
                nc.scalar.activation(tg[:], p[:, G_G:G_G + RC], Tanh,
                                     scale=1.0 / 64.0)
                t1 = gpool.tile([HB, RC], F32, name=f"t1{layer}{half}",
                                tag=f"t1{layer}{half}")
                nc.vector.tensor_mul(t1[:], si[:], tg[:])
                t2 = gpool.tile([HB, RC], F32, name=f"t2{layer}{half}",
                                tag=f"t2{layer}{half}")
                nc.vector.tensor_mul(t2[:], sf[:], c_old[:])
                cnew = cpool.tile([HB, RC], F32, name=f"c{layer}{half}",
                                  tag=f"c{layer}{half}")
                nc.vector.tensor_add(cnew[:], t1[:], t2[:])
                so = gpool.tile([HB, RC], F32, name=f"so{layer}{half}",
                                tag=f"so{layer}{half}")
                nc.scalar.activation(so[:], p[:, G_O:G_O + RC], Sig,
                                     scale=1.0 / 64.0)
                tcc = gpool.tile([HB, RC], F32, name=f"tc{layer}{half}",
                                 tag=f"tc{layer}{half}")
                nc.scalar.activation(tcc[:], cnew[:], Tanh)
                hn = gpool.tile([HB, RC], BF16, name=f"h{layer}n{half}",
                                tag=f"h{layer}n{half}")
                nc.vector.tensor_mul(hn[:], so[:], tcc[:])
                return hn, cnew

            def transpose_h(layer, hns):
                """PE-transpose the two [batch-half, dim] h tiles into one
                SBUF tile [dim 128, batch 256] (h^T orientation for the AG)."""
                tp = tppool.tile([RC, B], BF16, name=f"tp{layer}",
                                 tag=f"tp{layer}")
                for h in range(2):
                    nc.tensor.transpose(tp[:, h * HB : h * HB + HB],
                                        hns[h][:], ident[:])
                ts = gpool.tile([RC, B], BF16, name=f"tps{layer}",
                                tag=f"tps{layer}")
                nc.vector.tensor_copy(ts[:], tp[:])
                return ts

            def gather_launch(layer, tp):
                """Stage the h^T slice (PSUM) to local DRAM, kick AllGather."""
                ag_in = dramp.tile([RC, B], BF16, name=f"agin{layer}",
                                   tag=f"agin{layer}")
                nc.sync.dma_start(ag_in[:], tp[:])
                ag_out = dramp.tile([R, B], BF16, name=f"agout{layer}",
                                    tag=f"agout{layer}", addr_space="Shared")
                nc.gpsimd.collective_compute(
                    "AllGather", mybir.AluOpType.bypass,
                    replica_groups=[list(range(N_CORES))],
                    ins=[ag_in[:]], outs=[ag_out[:]],
                )
                return ag_out

            # ---- prologue ----
            h0T = load_h(0, h0T_d.ap())
            h1T = load_h(1, h1T_d.ap())
            c = {}
            for layer, cd in ((0, c0_d), (1, c1_d)):
                for h in range(2):
                    t = cpool.tile([HB, RC], F32, name=f"c{layer}{h}",
                                   tag=f"c{layer}{h}")
                    nc.sync.dma_start(t[:], cd.ap()[h * HB : h * HB + HB, :])
                    c[(layer, h)] = t

            wt = {0: load_wt(0), 1: load_wt(1 % U_RUN)}
            pre0 = alloc_pre(ps0, "pre0")
            bias_mm(pre0, bias0, 0)
            xpart_mms(pre0, wt[0])

            pre1 = None
            ag0_pend = None     # AG0(tau-1) DRAM tile, landed at body start
            ag1_pend = None     # AG1(tau-2) DRAM tile, landed mid-body
            wt_b_pending = {}   # widx for wt entries still missing group B
            for tau in range(T + 1):            # body T = layer-1 epilogue
                widx = tau % U_RUN
                pidx = (tau - 1) % U_RUN        # step gated for layer 1
                do_l0 = tau < T
                write_y = tau - 1 < U_RUN       # (tau>=1: y slot tau-1)

                _mark(nc, f"b{tau}:land0")
                if ag0_pend is not None:
                    h0T = load_h(0, ag0_pend[:])
                    ag0_pend = None
                # group-B chunks AFTER land0 on the sync queue, so a late
                # buffer-free sem can never head-of-line-block the land
                if tau + 1 in wt_b_pending:
                    load_wt_dmas(wt[tau + 1], wt_b_pending.pop(tau + 1),
                                 W_CHUNKS_B)

                # --- layer0: finish pre0(tau) per half, gate, AG0(tau) ---
                if do_l0:
                    _mark(nc, f"b{tau}:wh0")
                    part_mms(pre0, wt[tau], OFF_WH0, h0T, 0, True)
                    h0n = [None, None]
                    h0n[0], c[(0, 0)] = gate_cells(0, pre0, 0, c[(0, 0)])
                    part_mms(pre0, wt[tau], OFF_WH0, h0T, 1, True)
                    h0n[1], c[(0, 1)] = gate_cells(0, pre0, 1, c[(0, 1)])

                # --- layer1 part 1: wi1(tau-1) half A (PE filler while
                # gate0 finishes) ---
                _mark(nc, f"b{tau}:wi1")
                if pre1 is not None:
                    part_mms(pre1, wt[tau - 1], OFF_WI1, h0T, 0, False)

                if do_l0:
                    _mark(nc, f"b{tau}:tp0")
                    tp0 = transpose_h(0, h0n)
                    with tc.high_priority(offset=80):
                        ag0_pend = gather_launch(0, tp0)

                if pre1 is not None:
                    part_mms(pre1, wt[tau - 1], OFF_WI1, h0T, 1, False)

                load_next_w = do_l0 and tau < T - 1 and tau + 2 <= T - 1
                if load_next_w:
                    wt[tau + 2] = alloc_wt()
                    load_wt_dmas(wt[tau + 2], (tau + 2) % U_RUN, W_CHUNKS_A)
                    wt_b_pending[tau + 2] = (tau + 2) % U_RUN

                # --- land h1T(tau-2); wh1(tau-1); gate1; AG1(tau-1) ---
                if ag1_pend is not None:
                    h1T = load_h(1, ag1_pend[:])
                    ag1_pend = None
                if pre1 is not None:
                    _mark(nc, f"b{tau}:wh1")
                    part_mms(pre1, wt[tau - 1], OFF_WH1, h1T, 0, True)
                    h1n = [None, None]
                    h1n[0], c[(1, 0)] = gate_cells(1, pre1, 0, c[(1, 0)])
                    part_mms(pre1, wt[tau - 1], OFF_WH1, h1T, 1, True)
                    h1n[1], c[(1, 1)] = gate_cells(1, pre1, 1, c[(1, 1)])

                # --- fill: bias(es) + pre0(tau+1) x-part (PE filler while
                # gate1 finishes) ---
                if do_l0 and tau < T - 1:
                    _mark(nc, f"b{tau}:fill")
                    nidx = (tau + 1) % U_RUN
                    pre1 = alloc_pre(ps1, "pre1")
                    bias_mm(pre1, bias1, widx)
                    pre0 = alloc_pre(ps0, "pre0")
                    bias_mm(pre0, bias0, nidx)
                    xpart_mms(pre0, wt[tau + 1])
                    wt.pop(tau - 2, None)
                elif do_l0:
                    # last real body: only pre1(tau) remains (epilogue input)
                    pre1 = alloc_pre(ps1, "pre1")
                    bias_mm(pre1, bias1, widx)

                if pre1 is not None and tau >= 1:
                    _mark(nc, f"b{tau}:tp1")
                    if do_l0:
                        tp1 = transpose_h(1, h1n)
                        with tc.high_priority(offset=80):
                            ag1_pend = gather_launch(1, tp1)
                    if write_y:
                        for h in range(2):
                            nc.gpsimd.dma_start(
                                y_d.ap()[pidx][h * HB : h * HB + HB, :],
                                h1n[h][:])

            _mark(nc, "end")
    nc.compile()
    return nc


def prepare_in_maps(inputs: dict) -> list[dict]:
    import ml_dtypes
    bf = ml_dtypes.bfloat16
    f8 = ml_dtypes.float8_e4m3

    x = np.ascontiguousarray(np.asarray(inputs["x"], np.float32))
    st = np.asarray(inputs["init_states_input"], np.float32).reshape(B, 2 * L, R)
    h0i, c0i, h1i, c1i = st[:, 0], st[:, 1], st[:, 2], st[:, 3]

    xT = x.T.astype(bf)
    h0T = h0i.T.astype(bf)
    h1T = h1i.T.astype(bf)

    Wi0 = np.asarray(inputs["Wi0"], np.float32)[:U_RUN]
    Wh0 = np.asarray(inputs["Wh0"], np.float32)[:U_RUN]
    Wi1 = np.asarray(inputs["Wi1"], np.float32)[:U_RUN]
    Wh1 = np.asarray(inputs["Wh1"], np.float32)[:U_RUN]
    b0_full = (np.asarray(inputs["bi0"], np.float32)
               + np.asarray(inputs["bh0"], np.float32))[:U_RUN]
    b1_full = (np.asarray(inputs["bi1"], np.float32)
               + np.asarray(inputs["bh1"], np.float32))[:U_RUN]

    in_maps = []
    for k in range(N_CORES):
        rows = np.concatenate(
            [np.arange(g * R + k * RC, g * R + (k + 1) * RC) for g in range(4)])
        # moving weights, one tensor per step, partition-major. k-tile group
        # order matches in-body use: [Wh0 (8) | Wi1 (8) | Wh1 (8) | Wi0 (4)],
        # each k-tile block [128 k, W outdims] with outdims ordered [i|f|o|g].
        wk = np.concatenate(
            [Wh0[:, rows, :].transpose(0, 2, 1),
             Wi1[:, rows, :].transpose(0, 2, 1),
             Wh1[:, rows, :].transpose(0, 2, 1),
             Wi0[:, rows, :].transpose(0, 2, 1)], axis=1)
        wall = np.ascontiguousarray(
            wk.reshape(U_RUN, NKALL, 128, W).transpose(0, 2, 1, 3)
            .reshape(U_RUN, 128, NKALL * W) * 64.0).astype(f8)
        in_maps.append({
            "xT": xT,
            "h0T0": h0T,
            "h1T0": h1T,
            "c0i": np.ascontiguousarray(c0i[:, k * RC : (k + 1) * RC]),
            "c1i": np.ascontiguousarray(c1i[:, k * RC : (k + 1) * RC]),
            "wall": wall,
            "b0": np.ascontiguousarray(
                b0_full[:, rows].reshape(1, -1) * 64.0).astype(bf),
            "b1": np.ascontiguousarray(
                b1_full[:, rows].reshape(1, -1) * 64.0).astype(bf),
        })
    return in_maps


def assemble_output(inputs: dict, results: list[dict]) -> np.ndarray:
    st = np.asarray(inputs["init_states_input"], np.float32).reshape(B, 2 * L, R)
    h1i = st[:, 2]
    out = np.empty((B, U * R), np.float32)
    out[:, :R] = h1i
    for k in range(N_CORES):
        y = np.asarray(results[k]["y"], np.float32)  # [U_RUN, B, RC]
        for s in range(U_RUN):
            out[:, (s + 1) * R + k * RC : (s + 1) * R + (k + 1) * RC] = y[s]
    return out


PHASE_MARKS: list = []  # (label, first_instruction_id) — for profiling tools


def _mark(nc, label):
    PHASE_MARKS.append((label, nc.next_id()))


_CACHE: dict = {}


def _get_compiled():
    if "nc" not in _CACHE:
        _CACHE["nc"] = build_program(reps=1)
    return _CACHE["nc"]


def kernel(**inputs) -> np.ndarray:
    from concourse.bass_utils import run_bass_kernel_spmd

    nc = _get_compiled()
    in_maps = prepare_in_maps(inputs)
    res = run_bass_kernel_spmd(nc, in_maps, list(range(N_CORES)))
    return assemble_output(inputs, res.results)


# revision 10
# speedup vs baseline: 1.0764x; 1.0764x over previous
"""Trainium2 Bass kernel for nn_BuildLstmUnrollNet (2-layer LSTM, 11-step unroll,
per-step weights), distributed over 8 NeuronCores.

Strategy: 8-way tensor parallelism over the 4*R gate dimension. Each core owns a
128-row slice of each of the 4 gates (512 of 4096 pre-activation columns), so
per-step weight reads are fully disjoint across cores (HBM traffic /8). The
full batch (256) is kept on every core.

Matmuls run weight-stationary with the pre-activations TRANSPOSED: stationary
is a [K=128, 128] gate-chunk of the weight slice, moving is an h^T k-tile
[K=128, B=256], and PSUM holds pre^T = [gate-dims, batch]. Gating then happens
in [gate-dim, batch] layout, so the updated h slice comes out already in the
h^T orientation the AllGather needs.

Pipeline: layer 1 lags layer 0 by TWO steps, so in each body the layer-1
matmuls (wi1/wh1 of step tau-2) consume h0/h1 tiles gathered in EARLIER
bodies — they are wait-free PE work that fills the window while AG0(tau-1)
is still in flight, keeping the PE array continuously busy (HAM stays at
K=8/8) and hiding the collective latency. Each step still needs two 8-way
AllGathers (h0, h1); CC-queue order per body is AG1(tau-2) then AG0(tau).
h is bf16 end to end; y lands bf16 (upcast on the host); c stays fp32.
"""
import numpy as np

B, I, R, L, U = 256, 512, 1024, 2, 11
U_RUN = U - 1          # the 11th step never reaches the output
N_CORES = 8
RC = 128               # per-core rows per gate
W = 4 * RC             # per-core pre width (512)
NKX = I // 128         # 4 x k-tiles (layer0 input part)
NKH = R // 128         # 8 h k-tiles
NB = B // 128
# per-step weight k-tile groups, in use order
OFF_WI1 = 0
OFF_WH1 = 8
OFF_WH0 = 16
OFF_WI0 = 24
NKALL = 28
# gate column offsets inside a weight k-tile [128, W]
G_I, G_F, G_O, G_G = 0, 128, 256, 384


def build_program(reps: int = 1):
    import concourse.bacc as bacc
    import concourse.mybir as mybir
    import concourse.tile as tile

    F32 = mybir.dt.float32
    BF16 = mybir.dt.bfloat16
    FP8 = mybir.dt.float8e4
    Sig = mybir.ActivationFunctionType.Sigmoid
    Tanh = mybir.ActivationFunctionType.Tanh

    PHASE_MARKS.clear()
    nc = bacc.Bacc("TRN2", target_bir_lowering=False, debug=False,
                   num_devices=N_CORES)

    xT_d = nc.dram_tensor("xT", [I, B], BF16, kind="ExternalInput")
    h0T_d = nc.dram_tensor("h0T0", [R, B], BF16, kind="ExternalInput")
    h1T_d = nc.dram_tensor("h1T0", [R, B], BF16, kind="ExternalInput")
    c0_d = nc.dram_tensor("c0i", [RC, B], F32, kind="ExternalInput")
    c1_d = nc.dram_tensor("c1i", [RC, B], F32, kind="ExternalInput")
    # weights in scaled fp8-e4m3 (x64; compensated by ACT scale=1/64):
    # halves the dominant HBM/DMA-engine traffic
    wall_d = nc.dram_tensor("wall", [U_RUN, 128, NKALL * W], FP8,
                            kind="ExternalInput")
    b0_d = nc.dram_tensor("b0", [1, U_RUN * W], BF16, kind="ExternalInput")
    b1_d = nc.dram_tensor("b1", [1, U_RUN * W], BF16, kind="ExternalInput")
    # y[s] = h1 slice in [gate-dim, batch] orientation (host transposes)
    y_d = nc.dram_tensor("y", [U_RUN, RC, B], BF16, kind="ExternalOutput")

    T = reps * U_RUN

    with tile.TileContext(nc) as tc:
        with tc.tile_pool(name="const", bufs=1) as constp, \
             tc.tile_pool(name="h0pool", bufs=2) as h0pool, \
             tc.tile_pool(name="h1pool", bufs=2) as h1pool, \
             tc.tile_pool(name="cpool", bufs=2) as cpool, \
             tc.tile_pool(name="wpool", bufs=5) as wpool, \
             tc.tile_pool(name="gpool", bufs=2) as gpool, \
             tc.tile_pool(name="ps0", bufs=2, space="PSUM") as ps0, \
             tc.tile_pool(name="ps1", bufs=2, space="PSUM") as ps1, \
             tc.tile_pool(name="dramp", bufs=2, space="DRAM") as dramp:

            # ---- constants ----
            ones = constp.tile([1, B], BF16)
            nc.gpsimd.memset(ones[:], 1.0)
            xT_tiles = []
            for kk in range(NKX):
                xt = constp.tile([128, B], BF16, name=f"xT_{kk}")
                nc.sync.dma_start(xt[:], xT_d.ap()[kk * 128 : (kk + 1) * 128, :])
                xT_tiles.append(xt)
            bias0 = constp.tile([1, U_RUN * W], BF16)
            nc.sync.dma_start(bias0[:], b0_d.ap())
            bias1 = constp.tile([1, U_RUN * W], BF16)
            nc.sync.dma_start(bias1[:], b1_d.ap())

            # ---- helpers ----
            def load_h(layer, src_ap, nchunk=4):
                """[R, B] DRAM h^T -> one [128, NKH*B] tile (k-chunk kk at
                columns [kk*B, (kk+1)*B)), chunked so early-k consumers start
                sooner."""
                pool = h0pool if layer == 0 else h1pool
                t = pool.tile([128, NKH * B], BF16, name=f"h{layer}T",
                              tag=f"h{layer}T")
                step = NKH // nchunk
                for lo in range(0, NKH, step):
                    nc.sync.dma_start(
                        t[:, lo * B : (lo + step) * B]
                        .rearrange("p (k b) -> p k b", k=step),
                        src_ap[lo * 128 : (lo + step) * 128, :]
                        .rearrange("(k p) b -> p k b", p=128),
                    )
                return t

            # 4-k-tile chunks -> 4KB per-partition runs (full DMA packets)
            # group A: wi1 + wh1 (layer 1, needed first in-body)
            W_CHUNKS_A = [("wi1", OFF_WI1, 0, 4), ("wi1", OFF_WI1, 4, 8),
                          ("wh1", OFF_WH1, 0, 4), ("wh1", OFF_WH1, 4, 8)]
            W_CHUNKS_B = [("wh0", OFF_WH0, 0, 4), ("wh0", OFF_WH0, 4, 8),
                          ("wi0", OFF_WI0, 0, 4)]

            def alloc_wt():
                return {key: wpool.tile([128, n * W], FP8, name=f"{key}_s",
                                        tag=f"{key}_s")
                        for key, n in (("wi1", NKH), ("wh1", NKH),
                                       ("wh0", NKH), ("wi0", NKX))}

            def load_wt_dmas(tiles, widx, chunks):
                src = wall_d.ap()[widx]
                for key, off, lo, hi in chunks:
                    nc.sync.dma_start(
                        tiles[key][:, lo * W : hi * W],
                        src[:, (off + lo) * W : (off + hi) * W])

            def load_wt(widx):
                tiles = alloc_wt()
                load_wt_dmas(tiles, widx, W_CHUNKS_A)
                load_wt_dmas(tiles, widx, W_CHUNKS_B)
                return tiles

            def hstat(h_s, kk):
                return h_s[:, kk * B : (kk + 1) * B]

            def mm(out_ap, lhsT, rhs, start, stop):
                nc.tensor.matmul(out_ap, lhsT, rhs, start=start, stop=stop)

            # pre tiles: 2 gates per PSUM bank ([128, 2B] f32 = one 2KB bank);
            # each [128, B] gate region stays within its bank for matmul out
            def alloc_pre(pool, tag):
                ta = pool.tile([128, 2 * B], F32, name=f"{tag}a", tag=f"{tag}a")
                tb = pool.tile([128, 2 * B], F32, name=f"{tag}b", tag=f"{tag}b")
                return [ta[:, 0:B], ta[:, B : 2 * B],
                        tb[:, 0:B], tb[:, B : 2 * B]]

            # (tile-idx, weight-col) per gate, in compute order:
            # i, f first, then g, then o
            GATES = [(0, G_I), (1, G_F), (2, G_G), (3, G_O)]

            def pre_region(pre, gi):
                ti, _ = GATES[gi]
                return pre[ti]

            def bias_mms(pre, bias_t, widx):
                for gi, (ti, wc) in enumerate(GATES):
                    mm(pre_region(pre, gi),
                       bias_t[:, widx * W + wc : widx * W + wc + 128],
                       ones[:], True, False)

            def part_mms(pre, wtile, h_s, stop):
                """One weight part (8 k-tiles) accumulated into pre^T.
                Order: i,f over k-pairs first (consumes h k-tiles as the
                chunked land delivers them), then g, then o — so the first
                matmul starts ~1 land-chunk after the gather completes and
                the gate chain starts before o finishes."""
                for kc in range(4):
                    for gi in (0, 1):
                        _, wc = GATES[gi]
                        for kh in (2 * kc, 2 * kc + 1):
                            mm(pre_region(pre, gi),
                               wtile[:, kh * W + wc : kh * W + wc + 128],
                               hstat(h_s, kh), False,
                               stop and kh == NKH - 1)
                for gi in (2, 3):
                    _, wc = GATES[gi]
                    for kh in range(NKH):
                        mm(pre_region(pre, gi),
                           wtile[:, kh * W + wc : kh * W + wc + 128],
                           hstat(h_s, kh), False, stop and kh == NKH - 1)

            def xpart_mms(pre, wi0, widx):
                for gi, (ti, wc) in enumerate(GATES):
                    for kx in range(NKX):
                        mm(pre_region(pre, gi),
                           wi0[:, kx * W + wc : kx * W + wc + 128],
                           xT_tiles[kx][:], False, False)

            def gate_cells(layer, pre, c_old):
                """LSTM cell elementwise in [gate-dim, batch] layout; returns
                (h bf16 [128, B], c_new f32 [128, B]). pre = [i, f, g, o]."""
                si = gpool.tile([128, B], F32, name=f"si{layer}",
                                tag=f"si{layer}")
                nc.scalar.activation(si[:], pre[0], Sig, scale=1.0 / 64.0)
                sf = gpool.tile([128, B], F32, name=f"sf{layer}",
                                tag=f"sf{layer}")
                nc.scalar.activation(sf[:], pre[1], Sig, scale=1.0 / 64.0)
                tg = gpool.tile([128, B], F32, name=f"tg{layer}",
                                tag=f"tg{layer}")
                nc.scalar.activation(tg[:], pre[2], Tanh, scale=1.0 / 64.0)
                t1 = gpool.tile([128, B], F32, name=f"t1{layer}",
                                tag=f"t1{layer}")
                nc.vector.tensor_mul(t1[:], si[:], tg[:])
                t2 = gpool.tile([128, B], F32, name=f"t2{layer}",
                                tag=f"t2{layer}")
                nc.vector.tensor_mul(t2[:], sf[:], c_old[:])
                cnew = cpool.tile([128, B], F32, name=f"c{layer}",
                                  tag=f"c{layer}")
                nc.vector.tensor_add(cnew[:], t1[:], t2[:])
                so = gpool.tile([128, B], F32, name=f"so{layer}",
                                tag=f"so{layer}")
                nc.scalar.activation(so[:], pre[3], Sig, scale=1.0 / 64.0)
                tcc = gpool.tile([128, B], F32, name=f"tc{layer}",
                                 tag=f"tc{layer}")
                nc.scalar.activation(tcc[:], cnew[:], Tanh)
                hn = gpool.tile([128, B], BF16, name=f"h{layer}new",
                                tag=f"h{layer}new")
                nc.vector.tensor_mul(hn[:], so[:], tcc[:])
                return hn, cnew

            def gather_launch(layer, hn):
                """Stage the (already h^T-oriented) bf16 h slice to local DRAM
                and kick the AllGather."""
                ag_in = dramp.tile([128, B], BF16, name=f"agin{layer}",
                                   tag=f"agin{layer}")
                nc.sync.dma_start(ag_in[:], hn[:])
                ag_out = dramp.tile([R, B], BF16, name=f"agout{layer}",
                                    tag=f"agout{layer}", addr_space="Shared")
                nc.gpsimd.collective_compute(
                    "AllGather", mybir.AluOpType.bypass,
                    replica_groups=[list(range(N_CORES))],
                    ins=[ag_in[:]], outs=[ag_out[:]],
                )
                return ag_out

            # ---- prologue ----
            h0T_cur = load_h(0, h0T_d.ap())   # h0 before step 0
            h1T = load_h(1, h1T_d.ap())
            c = {}
            for layer, cd in ((0, c0_d), (1, c1_d)):
                t = cpool.tile([128, B], F32, name=f"c{layer}",
                               tag=f"c{layer}")
                nc.sync.dma_start(t[:], cd.ap())
                c[layer] = t

            wt = {0: load_wt(0), 1: load_wt(1 % U_RUN)}
            pre0 = alloc_pre(ps0, "pre0")
            bias_mms(pre0, bias0, 0)
            xpart_mms(pre0, wt[0]["wi0"], 0)

            # lag-2 pipeline state
            h0T_prev = None     # gathered h0 used by wi1 (one body older)
            pre1 = None         # pre1 of step tau-1, bias'd in body tau
            pre1_ready = None   # pre1 of step tau-2, to consume this body
            ag0_pend = None     # AG0(tau-1) DRAM tile, lands this body
            ag1_pend = None     # AG1(tau-3) DRAM tile, lands this body
            wt_b_pending = {}
            for tau in range(T + 2):
                widx = tau % U_RUN
                pidx = (tau - 2) % U_RUN        # step gated for layer 1
                do_l0 = tau < T
                do_l1 = tau >= 2                # layer 1 for step tau-2
                write_y = 0 <= tau - 2 < U_RUN

                # ---- sync-queue order: land1 (ready now), weight chunks,
                # then land0 (waits for AG0(tau-1) mid-body) ----
                _mark(nc, f"b{tau}:land1")
                if ag1_pend is not None:
                    h1T = load_h(1, ag1_pend[:])
                    ag1_pend = None
                if tau + 1 in wt_b_pending:
                    load_wt_dmas(wt[tau + 1], wt_b_pending.pop(tau + 1),
                                 W_CHUNKS_B)
                load_next_w = do_l0 and tau + 2 <= T - 1
                if load_next_w:
                    wt[tau + 2] = alloc_wt()
                    load_wt_dmas(wt[tau + 2], (tau + 2) % U_RUN, W_CHUNKS_A)
                    wt_b_pending[tau + 2] = (tau + 2) % U_RUN
                _mark(nc, f"b{tau}:land0")
                h0T_prev = h0T_cur
                if ag0_pend is not None:
                    h0T_cur = load_h(0, ag0_pend[:])
                    ag0_pend = None

                # ---- layer1 (step tau-2): wait-free PE work ----
                if do_l1:
                    _mark(nc, f"b{tau}:wi1")
                    part_mms(pre1_ready, wt[tau - 2]["wi1"], h0T_prev, False)
                    _mark(nc, f"b{tau}:wh1")
                    part_mms(pre1_ready, wt[tau - 2]["wh1"], h1T, True)
                    _mark(nc, f"b{tau}:gate1")
                    h1new, c[1] = gate_cells(1, pre1_ready, c[1])
                    if tau <= T:      # h1 after step tau-2, for wh1(tau-1)
                        with tc.high_priority(offset=80):
                            ag1_pend = gather_launch(1, h1new)
                    if write_y:
                        nc.gpsimd.dma_start(y_d.ap()[pidx], h1new[:])

                # ---- fill: biases + pre0(tau+1) x-part (PE filler while
                # gate1 runs and AG0(tau-1)/land0 complete) ----
                _mark(nc, f"b{tau}:fill")
                if 1 <= tau <= T:
                    # pre1 for step tau-1, consumed in body tau+1
                    pre1 = alloc_pre(ps1, "pre1")
                    bias_mms(pre1, bias1, (tau - 1) % U_RUN)
                else:
                    pre1 = None
                if do_l0 and tau < T - 1:
                    nidx = (tau + 1) % U_RUN
                    npre0 = alloc_pre(ps0, "pre0")
                    bias_mms(npre0, bias0, nidx)
                    xpart_mms(npre0, wt[tau + 1]["wi0"], nidx)
                else:
                    npre0 = None
                wt.pop(tau - 3, None)

                # ---- layer0: finish pre0(tau), gate, stage AG0(tau) ----
                if do_l0:
                    _mark(nc, f"b{tau}:wh0")
                    part_mms(pre0, wt[tau]["wh0"], h0T_cur, True)
                    _mark(nc, f"b{tau}:gate0")
                    h0new, c[0] = gate_cells(0, pre0, c[0])
                    with tc.high_priority(offset=80):
                        ag0_pend = gather_launch(0, h0new)
                    pre0 = npre0
                pre1_ready = pre1

            _mark(nc, "end")
    nc.compile()
    return nc


def prepare_in_maps(inputs: dict) -> list[dict]:
    import ml_dtypes
    bf = ml_dtypes.bfloat16
    f8 = ml_dtypes.float8_e4m3

    x = np.ascontiguousarray(np.asarray(inputs["x"], np.float32))
    st = np.asarray(inputs["init_states_input"], np.float32).reshape(B, 2 * L, R)
    h0i, c0i, h1i, c1i = st[:, 0], st[:, 1], st[:, 2], st[:, 3]

    xT = x.T.astype(bf)
    h0T = h0i.T.astype(bf)
    h1T = h1i.T.astype(bf)

    Wi0 = np.asarray(inputs["Wi0"], np.float32)[:U_RUN]
    Wh0 = np.asarray(inputs["Wh0"], np.float32)[:U_RUN]
    Wi1 = np.asarray(inputs["Wi1"], np.float32)[:U_RUN]
    Wh1 = np.asarray(inputs["Wh1"], np.float32)[:U_RUN]
    b0_full = (np.asarray(inputs["bi0"], np.float32)
               + np.asarray(inputs["bh0"], np.float32))[:U_RUN]
    b1_full = (np.asarray(inputs["bi1"], np.float32)
               + np.asarray(inputs["bh1"], np.float32))[:U_RUN]

    in_maps = []
    for k in range(N_CORES):
        rows = np.concatenate(
            [np.arange(g * R + k * RC, g * R + (k + 1) * RC) for g in range(4)])
        # moving weights, one tensor per step, partition-major. k-tile group
        # order matches in-body use: [Wi1 (8) | Wh1 (8) | Wh0 (8) | Wi0 (4)],
        # each k-tile [128, W] with gate chunks [i|f|o|g]... (column order
        # i,f,o,g matches `rows`).
        wk = np.concatenate(
            [Wi1[:, rows, :].transpose(0, 2, 1),
             Wh1[:, rows, :].transpose(0, 2, 1),
             Wh0[:, rows, :].transpose(0, 2, 1),
             Wi0[:, rows, :].transpose(0, 2, 1)], axis=1)
        wall = np.ascontiguousarray(
            wk.reshape(U_RUN, NKALL, 128, W).transpose(0, 2, 1, 3)
            .reshape(U_RUN, 128, NKALL * W) * 64.0).astype(f8)
        in_maps.append({
            "xT": xT,
            "h0T0": h0T,
            "h1T0": h1T,
            "c0i": np.ascontiguousarray(c0i[:, k * RC : (k + 1) * RC].T),
            "c1i": np.ascontiguousarray(c1i[:, k * RC : (k + 1) * RC].T),
            "wall": wall,
            "b0": np.ascontiguousarray(
                b0_full[:, rows].reshape(1, -1) * 64.0).astype(bf),
            "b1": np.ascontiguousarray(
                b1_full[:, rows].reshape(1, -1) * 64.0).astype(bf),
        })
    return in_maps


def assemble_output(inputs: dict, results: list[dict]) -> np.ndarray:
    st = np.asarray(inputs["init_states_input"], np.float32).reshape(B, 2 * L, R)
    h1i = st[:, 2]
    out = np.empty((B, U * R), np.float32)
    out[:, :R] = h1i
    for k in range(N_CORES):
        y = np.asarray(results[k]["y"], np.float32)  # [U_RUN, RC, B]
        for s in range(U_RUN):
            out[:, (s + 1) * R + k * RC : (s + 1) * R + (k + 1) * RC] = y[s].T
    return out


PHASE_MARKS: list = []  # (label, first_instruction_id) — for profiling tools


def _mark(nc, label):
    PHASE_MARKS.append((label, nc.next_id()))


_CACHE: dict = {}


def _get_compiled():
    if "nc" not in _CACHE:
        _CACHE["nc"] = build_program(reps=1)
    return _CACHE["nc"]


def kernel(**inputs) -> np.ndarray:
    from concourse.bass_utils import run_bass_kernel_spmd

    nc = _get_compiled()
    in_maps = prepare_in_maps(inputs)
    res = run_bass_kernel_spmd(nc, in_maps, list(range(N_CORES)))
    return assemble_output(inputs, res.results)


# revision 11
# speedup vs baseline: 1.0984x; 1.0205x over previous
"""Trainium2 Bass kernel for nn_BuildLstmUnrollNet (2-layer LSTM, 11-step unroll,
per-step weights), distributed over 8 NeuronCores.

Strategy: 8-way tensor parallelism over the 4*R gate dimension. Each core owns a
128-row slice of each of the 4 gates (512 of 4096 pre-activation columns), so
per-step weight reads are fully disjoint across cores (HBM traffic /8). The
full batch (256) is kept on every core.

Matmuls run weight-stationary with the pre-activations TRANSPOSED: stationary
is a [K=128, 128] gate-chunk of the weight slice, moving is an h^T k-tile
[K=128, B=256], and PSUM holds pre^T = [gate-dims, batch]. Gating then happens
in [gate-dim, batch] layout, so the updated h slice comes out already in the
h^T orientation the AllGather needs.

Pipeline: layer 1 lags layer 0 by TWO steps, so in each body the layer-1
matmuls (wi1/wh1 of step tau-2) consume h0/h1 tiles gathered in EARLIER
bodies — they are wait-free PE work that fills the window while AG0(tau-1)
is still in flight, keeping the PE array continuously busy (HAM stays at
K=8/8) and hiding the collective latency. Each step still needs two 8-way
AllGathers (h0, h1); CC-queue order per body is AG1(tau-2) then AG0(tau).
h is bf16 end to end; y lands bf16 (upcast on the host); c stays fp32.
"""
import numpy as np

B, I, R, L, U = 256, 512, 1024, 2, 11
U_RUN = U - 1          # the 11th step never reaches the output
N_CORES = 8
RC = 128               # per-core rows per gate
W = 4 * RC             # per-core pre width (512)
NKX = I // 128         # 4 x k-tiles (layer0 input part)
NKH = R // 128         # 8 h k-tiles
NB = B // 128
# per-step weight k-tile groups, in use order
OFF_WI1 = 0
OFF_WH1 = 8
OFF_WH0 = 16
OFF_WI0 = 24
NKALL = 28
# gate column offsets inside a weight k-tile [128, W]
G_I, G_F, G_O, G_G = 0, 128, 256, 384


def build_program(reps: int = 1):
    import concourse.bacc as bacc
    import concourse.mybir as mybir
    import concourse.tile as tile

    F32 = mybir.dt.float32
    BF16 = mybir.dt.bfloat16
    FP8 = mybir.dt.float8e4
    Sig = mybir.ActivationFunctionType.Sigmoid
    Tanh = mybir.ActivationFunctionType.Tanh

    PHASE_MARKS.clear()
    nc = bacc.Bacc("TRN2", target_bir_lowering=False, debug=False,
                   num_devices=N_CORES)

    xT_d = nc.dram_tensor("xT", [I, B], BF16, kind="ExternalInput")
    h0T_d = nc.dram_tensor("h0T0", [R, B], BF16, kind="ExternalInput")
    h1T_d = nc.dram_tensor("h1T0", [R, B], BF16, kind="ExternalInput")
    c0_d = nc.dram_tensor("c0i", [RC, B], F32, kind="ExternalInput")
    c1_d = nc.dram_tensor("c1i", [RC, B], F32, kind="ExternalInput")
    # weights in scaled fp8-e4m3 (x64; compensated by ACT scale=1/64):
    # halves the dominant HBM/DMA-engine traffic
    wall_d = nc.dram_tensor("wall", [U_RUN, 128, NKALL * W], FP8,
                            kind="ExternalInput")
    b0_d = nc.dram_tensor("b0", [1, U_RUN * W], BF16, kind="ExternalInput")
    b1_d = nc.dram_tensor("b1", [1, U_RUN * W], BF16, kind="ExternalInput")
    # y[s] = h1 slice in [gate-dim, batch] orientation (host transposes)
    y_d = nc.dram_tensor("y", [U_RUN, RC, B], BF16, kind="ExternalOutput")

    T = reps * U_RUN

    with tile.TileContext(nc) as tc:
        with tc.tile_pool(name="const", bufs=1) as constp, \
             tc.tile_pool(name="h0pool", bufs=2) as h0pool, \
             tc.tile_pool(name="h1pool", bufs=2) as h1pool, \
             tc.tile_pool(name="cpool", bufs=2) as cpool, \
             tc.tile_pool(name="wpool", bufs=5) as wpool, \
             tc.tile_pool(name="gpool", bufs=2) as gpool, \
             tc.tile_pool(name="ps0", bufs=2, space="PSUM") as ps0, \
             tc.tile_pool(name="ps1", bufs=2, space="PSUM") as ps1, \
             tc.tile_pool(name="dramp", bufs=2, space="DRAM") as dramp:

            # ---- constants ----
            ones = constp.tile([1, B], BF16)
            nc.gpsimd.memset(ones[:], 1.0)
            xT_tiles = []
            for kk in range(NKX):
                xt = constp.tile([128, B], BF16, name=f"xT_{kk}")
                nc.sync.dma_start(xt[:], xT_d.ap()[kk * 128 : (kk + 1) * 128, :])
                xT_tiles.append(xt)
            bias0 = constp.tile([1, U_RUN * W], BF16)
            nc.sync.dma_start(bias0[:], b0_d.ap())
            bias1 = constp.tile([1, U_RUN * W], BF16)
            nc.sync.dma_start(bias1[:], b1_d.ap())

            # ---- helpers ----
            def load_h(layer, src_ap, nchunk=4):
                """[R, B] DRAM h^T -> one [128, NKH*B] tile (k-chunk kk at
                columns [kk*B, (kk+1)*B)), chunked so early-k consumers start
                sooner."""
                pool = h0pool if layer == 0 else h1pool
                t = pool.tile([128, NKH * B], BF16, name=f"h{layer}T",
                              tag=f"h{layer}T")
                step = NKH // nchunk
                for lo in range(0, NKH, step):
                    nc.sync.dma_start(
                        t[:, lo * B : (lo + step) * B]
                        .rearrange("p (k b) -> p k b", k=step),
                        src_ap[lo * 128 : (lo + step) * 128, :]
                        .rearrange("(k p) b -> p k b", p=128),
                    )
                return t

            # 4-k-tile chunks -> 4KB per-partition runs (full DMA packets)
            # group A: wi1 + wh1 (layer 1, needed first in-body)
            W_CHUNKS_A = [("wi1", OFF_WI1, 0, 4), ("wi1", OFF_WI1, 4, 8),
                          ("wh1", OFF_WH1, 0, 4), ("wh1", OFF_WH1, 4, 8)]
            W_CHUNKS_B = [("wh0", OFF_WH0, 0, 4), ("wh0", OFF_WH0, 4, 8),
                          ("wi0", OFF_WI0, 0, 4)]

            def alloc_wt():
                return {key: wpool.tile([128, n * W], FP8, name=f"{key}_s",
                                        tag=f"{key}_s")
                        for key, n in (("wi1", NKH), ("wh1", NKH),
                                       ("wh0", NKH), ("wi0", NKX))}

            def load_wt_dmas(tiles, widx, chunks):
                src = wall_d.ap()[widx]
                for key, off, lo, hi in chunks:
                    nc.sync.dma_start(
                        tiles[key][:, lo * W : hi * W],
                        src[:, (off + lo) * W : (off + hi) * W])

            def load_wt(widx):
                tiles = alloc_wt()
                load_wt_dmas(tiles, widx, W_CHUNKS_A)
                load_wt_dmas(tiles, widx, W_CHUNKS_B)
                return tiles

            def hstat(h_s, kk):
                return h_s[:, kk * B : (kk + 1) * B]

            def mm(out_ap, lhsT, rhs, start, stop):
                nc.tensor.matmul(out_ap, lhsT, rhs, start=start, stop=stop)

            # pre tiles: 2 gates per PSUM bank ([128, 2B] f32 = one 2KB bank);
            # each [128, B] gate region stays within its bank for matmul out
            def alloc_pre(pool, tag):
                ta = pool.tile([128, 2 * B], F32, name=f"{tag}a", tag=f"{tag}a")
                tb = pool.tile([128, 2 * B], F32, name=f"{tag}b", tag=f"{tag}b")
                return [ta[:, 0:B], ta[:, B : 2 * B],
                        tb[:, 0:B], tb[:, B : 2 * B]]

            # (tile-idx, weight-col) per gate, in compute order:
            # i, f first, then g, then o
            GATES = [(0, G_I), (1, G_F), (2, G_G), (3, G_O)]

            def pre_region(pre, gi):
                ti, _ = GATES[gi]
                return pre[ti]

            def bias_mms(pre, bias_t, widx):
                for gi, (ti, wc) in enumerate(GATES):
                    mm(pre_region(pre, gi),
                       bias_t[:, widx * W + wc : widx * W + wc + 128],
                       ones[:], True, False)

            def part_mms(pre, wtile, h_s, stop):
                """One weight part (8 k-tiles) accumulated into pre^T.
                Order: i,f over k-pairs first (consumes h k-tiles as the
                chunked land delivers them), then g, then o — so the first
                matmul starts ~1 land-chunk after the gather completes and
                the gate chain starts before o finishes."""
                for kc in range(4):
                    for gi in (0, 1):
                        _, wc = GATES[gi]
                        for kh in (2 * kc, 2 * kc + 1):
                            mm(pre_region(pre, gi),
                               wtile[:, kh * W + wc : kh * W + wc + 128],
                               hstat(h_s, kh), False,
                               stop and kh == NKH - 1)
                for gi in (2, 3):
                    _, wc = GATES[gi]
                    for kh in range(NKH):
                        mm(pre_region(pre, gi),
                           wtile[:, kh * W + wc : kh * W + wc + 128],
                           hstat(h_s, kh), False, stop and kh == NKH - 1)

            def xpart_mms(pre, wi0, widx):
                for gi, (ti, wc) in enumerate(GATES):
                    for kx in range(NKX):
                        mm(pre_region(pre, gi),
                           wi0[:, kx * W + wc : kx * W + wc + 128],
                           xT_tiles[kx][:], False, False)

            def gate_cells(layer, pre, c_old):
                """LSTM cell elementwise in [gate-dim, batch] layout; returns
                (h bf16 [128, B], c_new f32 [128, B]). pre = [i, f, g, o]."""
                si = gpool.tile([128, B], F32, name=f"si{layer}",
                                tag=f"si{layer}")
                nc.scalar.activation(si[:], pre[0], Sig, scale=1.0 / 64.0)
                sf = gpool.tile([128, B], F32, name=f"sf{layer}",
                                tag=f"sf{layer}")
                nc.scalar.activation(sf[:], pre[1], Sig, scale=1.0 / 64.0)
                tg = gpool.tile([128, B], F32, name=f"tg{layer}",
                                tag=f"tg{layer}")
                nc.scalar.activation(tg[:], pre[2], Tanh, scale=1.0 / 64.0)
                t1 = gpool.tile([128, B], F32, name=f"t1{layer}",
                                tag=f"t1{layer}")
                nc.vector.tensor_mul(t1[:], si[:], tg[:])
                t2 = gpool.tile([128, B], F32, name=f"t2{layer}",
                                tag=f"t2{layer}")
                nc.vector.tensor_mul(t2[:], sf[:], c_old[:])
                cnew = cpool.tile([128, B], F32, name=f"c{layer}",
                                  tag=f"c{layer}")
                nc.vector.tensor_add(cnew[:], t1[:], t2[:])
                so = gpool.tile([128, B], F32, name=f"so{layer}",
                                tag=f"so{layer}")
                nc.scalar.activation(so[:], pre[3], Sig, scale=1.0 / 64.0)
                tcc = gpool.tile([128, B], F32, name=f"tc{layer}",
                                 tag=f"tc{layer}")
                nc.scalar.activation(tcc[:], cnew[:], Tanh)
                hn = gpool.tile([128, B], BF16, name=f"h{layer}new",
                                tag=f"h{layer}new")
                nc.vector.tensor_mul(hn[:], so[:], tcc[:])
                return hn, cnew

            def gather_launch(layer, hn):
                """Stage the (already h^T-oriented) bf16 h slice to local DRAM
                and kick the AllGather."""
                ag_in = dramp.tile([128, B], BF16, name=f"agin{layer}",
                                   tag=f"agin{layer}")
                nc.sync.dma_start(ag_in[:], hn[:])
                ag_out = dramp.tile([R, B], BF16, name=f"agout{layer}",
                                    tag=f"agout{layer}", addr_space="Shared")
                nc.gpsimd.collective_compute(
                    "AllGather", mybir.AluOpType.bypass,
                    replica_groups=[list(range(N_CORES))],
                    ins=[ag_in[:]], outs=[ag_out[:]],
                )
                return ag_out

            # ---- prologue ----
            h0T_cur = load_h(0, h0T_d.ap())   # h0 before step 0
            h1T = load_h(1, h1T_d.ap())
            c = {}
            for layer, cd in ((0, c0_d), (1, c1_d)):
                t = cpool.tile([128, B], F32, name=f"c{layer}",
                               tag=f"c{layer}")
                nc.sync.dma_start(t[:], cd.ap())
                c[layer] = t

            wt = {0: load_wt(0), 1: load_wt(1 % U_RUN)}
            pre0 = alloc_pre(ps0, "pre0")
            bias_mms(pre0, bias0, 0)
            xpart_mms(pre0, wt[0]["wi0"], 0)

            # lag-2 pipeline state
            h0T_prev = None     # gathered h0 used by wi1 (one body older)
            pre1 = None         # pre1 of step tau-1, bias'd in body tau
            pre1_ready = None   # pre1 of step tau-2, to consume this body
            ag0_pend = None     # AG0(tau-1) DRAM tile, lands this body
            ag1_pend = None     # AG1(tau-3) DRAM tile, lands this body
            wt_b_pending = {}
            for tau in range(T + 2):
                widx = tau % U_RUN
                pidx = (tau - 2) % U_RUN        # step gated for layer 1
                do_l0 = tau < T
                do_l1 = tau >= 2                # layer 1 for step tau-2
                write_y = 0 <= tau - 2 < U_RUN

                # ---- sync-queue order: land1 (ready now), weight chunks,
                # then land0 (waits for AG0(tau-1) mid-body) ----
                _mark(nc, f"b{tau}:land1")
                if ag1_pend is not None:
                    h1T = load_h(1, ag1_pend[:])
                    ag1_pend = None
                if tau + 1 in wt_b_pending:
                    load_wt_dmas(wt[tau + 1], wt_b_pending.pop(tau + 1),
                                 W_CHUNKS_B)
                load_next_w = do_l0 and tau + 2 <= T - 1
                if load_next_w:
                    wt[tau + 2] = alloc_wt()
                    load_wt_dmas(wt[tau + 2], (tau + 2) % U_RUN, W_CHUNKS_A)
                    wt_b_pending[tau + 2] = (tau + 2) % U_RUN
                _mark(nc, f"b{tau}:land0")
                h0T_prev = h0T_cur
                if ag0_pend is not None:
                    h0T_cur = load_h(0, ag0_pend[:])
                    ag0_pend = None

                # ---- layer1 (step tau-2): wait-free PE work ----
                if do_l1:
                    _mark(nc, f"b{tau}:wi1")
                    part_mms(pre1_ready, wt[tau - 2]["wi1"], h0T_prev, False)
                    _mark(nc, f"b{tau}:wh1")
                    part_mms(pre1_ready, wt[tau - 2]["wh1"], h1T, True)
                    _mark(nc, f"b{tau}:gate1")
                    h1new, c[1] = gate_cells(1, pre1_ready, c[1])
                    if tau <= T:      # h1 after step tau-2, for wh1(tau-1)
                        with tc.high_priority(offset=80):
                            ag1_pend = gather_launch(1, h1new)
                    if write_y:
                        nc.gpsimd.dma_start(y_d.ap()[pidx], h1new[:])

                # ---- fill: biases + pre0(tau+1) x-part (PE filler while
                # gate1 runs and AG0(tau-1)/land0 complete) ----
                _mark(nc, f"b{tau}:fill")
                if 1 <= tau <= T:
                    # pre1 for step tau-1, consumed in body tau+1
                    pre1 = alloc_pre(ps1, "pre1")
                    bias_mms(pre1, bias1, (tau - 1) % U_RUN)
                else:
                    pre1 = None
                if do_l0 and tau < T - 1:
                    nidx = (tau + 1) % U_RUN
                    npre0 = alloc_pre(ps0, "pre0")
                    bias_mms(npre0, bias0, nidx)
                    xpart_mms(npre0, wt[tau + 1]["wi0"], nidx)
                else:
                    npre0 = None
                wt.pop(tau - 3, None)

                # ---- layer0: finish pre0(tau), gate, stage AG0(tau) ----
                if do_l0:
                    _mark(nc, f"b{tau}:wh0")
                    part_mms(pre0, wt[tau]["wh0"], h0T_cur, True)
                    _mark(nc, f"b{tau}:gate0")
                    h0new, c[0] = gate_cells(0, pre0, c[0])
                    with tc.high_priority(offset=80):
                        ag0_pend = gather_launch(0, h0new)
                    pre0 = npre0

                pre1_ready = pre1

            _mark(nc, "end")
    nc.compile()
    return nc


def prepare_in_maps(inputs: dict) -> list[dict]:
    import ml_dtypes
    bf = ml_dtypes.bfloat16
    f8 = ml_dtypes.float8_e4m3

    x = np.ascontiguousarray(np.asarray(inputs["x"], np.float32))
    st = np.asarray(inputs["init_states_input"], np.float32).reshape(B, 2 * L, R)
    h0i, c0i, h1i, c1i = st[:, 0], st[:, 1], st[:, 2], st[:, 3]

    xT = x.T.astype(bf)
    h0T = h0i.T.astype(bf)
    h1T = h1i.T.astype(bf)

    Wi0 = np.asarray(inputs["Wi0"], np.float32)[:U_RUN]
    Wh0 = np.asarray(inputs["Wh0"], np.float32)[:U_RUN]
    Wi1 = np.asarray(inputs["Wi1"], np.float32)[:U_RUN]
    Wh1 = np.asarray(inputs["Wh1"], np.float32)[:U_RUN]
    b0_full = (np.asarray(inputs["bi0"], np.float32)
               + np.asarray(inputs["bh0"], np.float32))[:U_RUN]
    b1_full = (np.asarray(inputs["bi1"], np.float32)
               + np.asarray(inputs["bh1"], np.float32))[:U_RUN]

    in_maps = []
    for k in range(N_CORES):
        rows = np.concatenate(
            [np.arange(g * R + k * RC, g * R + (k + 1) * RC) for g in range(4)])
        # moving weights, one tensor per step, partition-major. k-tile group
        # order matches in-body use: [Wi1 (8) | Wh1 (8) | Wh0 (8) | Wi0 (4)],
        # each k-tile [128, W] with gate chunks [i|f|o|g]... (column order
        # i,f,o,g matches `rows`).
        wk = np.concatenate(
            [Wi1[:, rows, :].transpose(0, 2, 1),
             Wh1[:, rows, :].transpose(0, 2, 1),
             Wh0[:, rows, :].transpose(0, 2, 1),
             Wi0[:, rows, :].transpose(0, 2, 1)], axis=1)
        wall = np.ascontiguousarray(
            wk.reshape(U_RUN, NKALL, 128, W).transpose(0, 2, 1, 3)
            .reshape(U_RUN, 128, NKALL * W) * 64.0).astype(f8)
        in_maps.append({
            "xT": xT,
            "h0T0": h0T,
            "h1T0": h1T,
            "c0i": np.ascontiguousarray(c0i[:, k * RC : (k + 1) * RC].T),
            "c1i": np.ascontiguousarray(c1i[:, k * RC : (k + 1) * RC].T),
            "wall": wall,
            "b0": np.ascontiguousarray(
                b0_full[:, rows].reshape(1, -1) * 64.0).astype(bf),
            "b1": np.ascontiguousarray(
                b1_full[:, rows].reshape(1, -1) * 64.0).astype(bf),
        })
    return in_maps


def assemble_output(inputs: dict, results: list[dict]) -> np.ndarray:
    st = np.asarray(inputs["init_states_input"], np.float32).reshape(B, 2 * L, R)
    h1i = st[:, 2]
    out = np.empty((B, U * R), np.float32)
    out[:, :R] = h1i
    for k in range(N_CORES):
        y = np.asarray(results[k]["y"], np.float32)  # [U_RUN, RC, B]
        for s in range(U_RUN):
            out[:, (s + 1) * R + k * RC : (s + 1) * R + (k + 1) * RC] = y[s].T
    return out


PHASE_MARKS: list = []  # (label, first_instruction_id) — for profiling tools


def _mark(nc, label):
    PHASE_MARKS.append((label, nc.next_id()))


_CACHE: dict = {}


def _get_compiled():
    if "nc" not in _CACHE:
        _CACHE["nc"] = build_program(reps=1)
    return _CACHE["nc"]


def kernel(**inputs) -> np.ndarray:
    from concourse.bass_utils import run_bass_kernel_spmd

    nc = _get_compiled()
    in_maps = prepare_in_maps(inputs)
    res = run_bass_kernel_spmd(nc, in_maps, list(range(N_CORES)))
    return assemble_output(inputs, res.results)
